# revision 16
# baseline (speedup 1.0000x reference)
"""ConvSelfAttention Trainium2 kernel.

Reference computation (B=4, C=512, N=2048, H=8 heads, D=64):
    qkv = w_qkv @ x          (pointwise conv == matmul over channels)
    per head: sim = (q*D^-.5)^T k ; attn = softmax(sim, axis=j)
    out = attn @ v^T ; y = w_out @ out_heads + b_out

Sharding: 8 cores = 4 batches x 2 head-groups (4 heads each). Each core
computes its batch's x-projections restricted to its 4 heads, runs
attention, and produces a partial output projection y_part[c, n]
(sum over its heads' hd columns of w_out). Host sums the two partials
per batch and adds the bias.

On-chip layout notes:
  - All matmuls take bf16 inputs (fp32 PSUM accumulation); fp32 matmul
    runs at half rate on the PE.
  - Attention is computed transposed: S^T[j, i] = k^T q so softmax's
    sum over j is a matmul contraction. exp() runs on ScalarE from
    PSUM in wide [128, 3*512] instructions. The softmax denominator
    comes from a ones-column appended to v^T (PV matmul row 64).
  - Normalization 1/l is broadcast across partitions with a rank-1
    matmul (ones ⊗ recip_l) since engines cannot partition-broadcast.
  - Heads are processed in pairs living in SBUF partitions 0-63/64-127
    so the K=64 S^T matmuls auto-derive tile_position (0,0)/(64,0) and
    run concurrently in the two 64-row PE tiles.
"""

import numpy as np
import ml_dtypes

B, C, N = 4, 512, 2048
H, D = 8, 64
HID = H * D
SCALE = D ** -0.5
NCORES = 8
HPC = 4          # heads per core
NT = 4           # i-tiles of 512
KT = 4           # k-tiles of 128 over C
JB = 16          # j-blocks of 128
SLOT = 512
SPW = 3          # S^T psum slots per tile (3 banks)

bf16 = ml_dtypes.bfloat16

_PROG = None


def _build_program():
    import concourse.mybir as mybir
    import concourse.tile as tile
    from concourse import bacc

    fp32 = mybir.dt.float32
    bfl = mybir.dt.bfloat16
    Exp = mybir.ActivationFunctionType.Exp

    nc = bacc.Bacc("TRN2", target_bir_lowering=False, debug=False)

    x_d = nc.dram_tensor("x", [C, N], bfl, kind="ExternalInput")
    wqk_d = nc.dram_tensor("wqk", [C, 512], bfl, kind="ExternalInput")
    wv_d = nc.dram_tensor("wv", [C, 256], bfl, kind="ExternalInput")
    wo_d = nc.dram_tensor("wo", [256, C], bfl, kind="ExternalInput")
    y_d = nc.dram_tensor("y", [C, N], fp32, kind="ExternalOutput")

    with tile.TileContext(nc) as tc:
        with (
            tc.tile_pool(name="const", bufs=1) as constp,
            tc.tile_pool(name="big", bufs=1) as bigp,
            tc.tile_pool(name="pt", bufs=24) as ptp,
            tc.tile_pool(name="ov", bufs=4) as ovp,
            tc.tile_pool(name="spsum", bufs=2, space="PSUM") as sp,
            tc.tile_pool(name="wpsum", bufs=2, space="PSUM") as wp,
        ):
            # ---- constants
            # ones2: block "selector" for the merged 1/l broadcast matmul:
            # lb = ones2.T @ lrow puts lrow row0 on partitions 0-63 and row1
            # on partitions 64-127.
            ones2 = constp.tile([128, 128], fp32, tag="ones", name="ones")
            nc.vector.memset(ones2[:], 0.0)
            nc.vector.memset(ones2[0:1, 0:64], 1.0)
            nc.vector.memset(ones2[64:65, 64:128], 1.0)
            lrow = constp.tile([128, SLOT], fp32, tag="lrow", name="lrow")
            nc.vector.memset(lrow[:], 0.0)

            # ---- input loads. Order matters for pipeline startup: the qk
            # projection's first psum group reads wqk[kt][:, 0:128] and
            # x[kt][:, 0:512] for all kt, so load weights first and x in
            # column chunks, chunk-major (subtile deps let the first matmuls
            # start after ~400KB instead of the full 3MB).
            wqk_sb = []
            for kt in range(KT):
                t = bigp.tile([128, 512], bfl, tag=f"wqk{kt}", name=f"wqk{kt}")
                nc.sync.dma_start(t[:], wqk_d[kt * 128:(kt + 1) * 128, :])
                wqk_sb.append(t)
            x_sb = [
                bigp.tile([128, N], bfl, tag=f"x{kt}", name=f"x{kt}")
                for kt in range(KT)
            ]
            for nt in range(NT):
                for kt in range(KT):
                    nc.sync.dma_start(
                        x_sb[kt][:, nt * SLOT:(nt + 1) * SLOT],
                        x_d[kt * 128:(kt + 1) * 128, nt * SLOT:(nt + 1) * SLOT],
                    )
            wv_sb = []
            for kt in range(KT):
                t = bigp.tile([128, 256], bfl, tag=f"wv{kt}", name=f"wv{kt}")
                nc.sync.dma_start(t[:], wv_d[kt * 128:(kt + 1) * 128, :])
                wv_sb.append(t)
            wo_sb = []
            for kt in range(2):
                t = bigp.tile([128, 512], bfl, tag=f"wo{kt}", name=f"wo{kt}")
                nc.sync.dma_start(t[:], wo_d[kt * 128:(kt + 1) * 128, :])
                wo_sb.append(t)

            # ---- QK projection -> q_sb[hp], k_sb[hp] (2 heads stacked per tile)
            # host column order: q-hp0 | k-hp0 | q-hp1 | k-hp1 so that hp0's
            # attention can start after only half the projection.
            q_sb = [bigp.tile([128, N], bfl, tag=f"q{hp}", name=f"q{hp}") for hp in range(2)]
            k_sb = [bigp.tile([128, N], bfl, tag=f"k{hp}", name=f"k{hp}") for hp in range(2)]
            dest = [q_sb[0], k_sb[0], q_sb[1], k_sb[1]]

            def emit_qkproj(mt, nts=range(NT)):
                for nt in nts:
                    ps = wp.tile([128, SLOT], fp32, tag="w", name="wps")
                    for kt in range(KT):
                        nc.tensor.matmul(
                            ps[:],
                            wqk_sb[kt][:, mt * 128:(mt + 1) * 128],
                            x_sb[kt][:, nt * SLOT:(nt + 1) * SLOT],
                            start=(kt == 0),
                            stop=(kt == KT - 1),
                        )
                    nc.vector.tensor_copy(
                        dest[mt][:, nt * SLOT:(nt + 1) * SLOT], ps[:]
                    )

            # ---- v^T projection, augmented with a ones column per head
            # vt_sb[jb] : [128 (j), HPC, 65] ; [:, h, 0:64] = v^T, [:, h, 64] = 1
            vt_sb = []

            def emit_vtproj():
                for jb in range(JB):
                    t = bigp.tile([128, HPC, 65], bfl, tag=f"vt{jb}", name=f"vt{jb}")
                    nc.vector.memset(t[:, :, 64:65], 1.0)
                    ps = wp.tile([128, SLOT], fp32, tag="w", name="wps")
                    for kt in range(KT):
                        nc.tensor.matmul(
                            ps[:, 0:256],
                            x_sb[kt][:, jb * 128:(jb + 1) * 128],
                            wv_sb[kt][:],
                            start=(kt == 0),
                            stop=(kt == KT - 1),
                        )
                    nc.vector.tensor_copy(
                        t[:, :, 0:64],
                        ps[:, 0:256].rearrange("p (h d) -> p h d", h=HPC),
                    )
                    vt_sb.append(t)

            # ---- attention, software-pipelined per (it, hp) group
            oh_sb = [bigp.tile([128, N], bfl, tag=f"oh{hp}", name=f"oh{hp}") for hp in range(2)]
            groups = [(it, hp) for it in range(NT) for hp in range(2)]

            def emit_sim_exp(it, hp):
                """S^T matmuls + exp for one (i-tile, head-pair). Returns the
                list of (pt_tile, [(slot_idx, hh, jb), ...])."""
                out = []
                slots = [(jb, hh) for jb in range(JB) for hh in range(2)]
                for base in range(0, len(slots), SPW):
                    chunk = slots[base:base + SPW]
                    st = sp.tile([128, SPW * SLOT], fp32, tag="s", name="st")
                    for si, (jb, hh) in enumerate(chunk):
                        lo, hi = hh * 64, hh * 64 + 64
                        nc.tensor.matmul(
                            st[:, si * SLOT:(si + 1) * SLOT],
                            k_sb[hp][lo:hi, jb * 128:(jb + 1) * 128],
                            q_sb[hp][lo:hi, it * SLOT:(it + 1) * SLOT],
                        )
                    w = len(chunk) * SLOT
                    pt = ptp.tile([128, SPW * SLOT], bfl, tag="pt", name="pt")
                    nc.scalar.activation(pt[:, 0:w], st[:, 0:w], Exp)
                    out.append((pt, chunk))
                return out

            def emit_pv_tail(it, hp, ptiles):
                pv = [wp.tile([128, SLOT], fp32, tag="w", name="pv") for _ in range(2)]
                nmm = [0, 0]
                for pt, chunk in ptiles:
                    for si, (jb, hh) in enumerate(chunk):
                        nc.tensor.matmul(
                            pv[hh][0:65, :],
                            vt_sb[jb][:, hp * 2 + hh, :],
                            pt[:, si * SLOT:(si + 1) * SLOT],
                            start=(nmm[hh] == 0),
                            stop=(nmm[hh] == JB - 1),
                        )
                        nmm[hh] += 1
                ovs = []
                for hh in range(2):
                    ov = ovp.tile([65, SLOT], fp32, tag="ov", name="ov")
                    nc.vector.tensor_copy(ov[:], pv[hh][0:65, :])
                    nc.vector.reciprocal(lrow[hh * 64:hh * 64 + 1, :], ov[64:65, :])
                    ovs.append(ov)
                # single broadcast matmul: lb rows 0-63 = 1/l_h0, 64-127 = 1/l_h1
                lb = wp.tile([128, SLOT], fp32, tag="w", name="lb")
                nc.tensor.matmul(lb[:], ones2[:], lrow[:])
                for hh in range(2):
                    nc.vector.tensor_mul(
                        oh_sb[hp][hh * 64:(hh + 1) * 64,
                                  it * SLOT:(it + 1) * SLOT],
                        ovs[hh][0:64, :],
                        lb[hh * 64:(hh + 1) * 64, :],
                    )

            def emit_outproj(it):
                for mt in range(4):
                    yp = wp.tile([128, SLOT], fp32, tag="w", name="yp")
                    for kt in range(2):
                        nc.tensor.matmul(
                            yp[:],
                            wo_sb[kt][:, mt * 128:(mt + 1) * 128],
                            oh_sb[kt][:, it * SLOT:(it + 1) * SLOT],
                            start=(kt == 0),
                            stop=(kt == 1),
                        )
                    ys = ovp.tile([128, SLOT], fp32, tag="ys", name="ys")
                    nc.vector.tensor_copy(ys[:], yp[:])
                    nc.sync.dma_start(
                        y_d[mt * 128:(mt + 1) * 128, it * SLOT:(it + 1) * SLOT],
                        ys[:],
                    )

            # Early start: project only what the first attention group needs
            # (q-hp0 i-tile 0 + all of k-hp0), launch it so ACT gets exp work
            # ASAP, then fill in the remaining projections while ACT chews on
            # group 0.
            emit_qkproj(0, nts=[0])
            emit_qkproj(1)
            first = emit_sim_exp(*groups[0])
            emit_qkproj(0, nts=[1, 2, 3])
            emit_qkproj(2)
            emit_qkproj(3)
            emit_vtproj()

            pending = (groups[0][0], groups[0][1], first)
            for it, hp in groups[1:]:
                ptiles = emit_sim_exp(it, hp)
                pit, php, pp = pending
                emit_pv_tail(pit, php, pp)
                if php == 1:
                    emit_outproj(pit)
                pending = (it, hp, ptiles)
            pit, php, pp = pending
            emit_pv_tail(pit, php, pp)
            emit_outproj(pit)

    nc.compile()
    return nc


def _get_program():
    global _PROG
    if _PROG is None:
        _PROG = _build_program()
    return _PROG


def _make_in_maps(x, w_qkv, w_out):
    xs = x.astype(bf16)
    in_maps = []
    for core in range(NCORES):
        b, g = divmod(core, 2)
        r0 = 256 * g
        wq = w_qkv[r0:r0 + 256, :] * SCALE
        wk = w_qkv[512 + r0:512 + r0 + 256, :]
        wv = w_qkv[1024 + r0:1024 + r0 + 256, :]
        wqkT = np.ascontiguousarray(
            np.concatenate(
                [wq[0:128], wk[0:128], wq[128:256], wk[128:256]], axis=0
            ).T.astype(bf16)
        )
        wvT = np.ascontiguousarray(wv.T.astype(bf16))
        woT = np.ascontiguousarray(w_out[:, r0:r0 + 256].T.astype(bf16))
        in_maps.append({
            "x": np.ascontiguousarray(xs[b]),
            "wqk": wqkT,
            "wv": wvT,
            "wo": woT,
        })
    return in_maps


def _gather(results, b_out):
    parts = [results[c]["y"] for c in range(NCORES)]
    y = np.stack([parts[2 * b] + parts[2 * b + 1] for b in range(B)])
    y += b_out[None, :, None]
    return y.astype(np.float32)


def kernel(x, w_qkv, w_out, b_out):
    from concourse.bass_utils import run_bass_kernel_spmd

    nc = _get_program()

    x = np.asarray(x, dtype=np.float32)
    w_qkv = np.asarray(w_qkv, dtype=np.float32)
    w_out = np.asarray(w_out, dtype=np.float32)
    b_out = np.asarray(b_out, dtype=np.float32)

    in_maps = _make_in_maps(x, w_qkv, w_out)
    res = run_bass_kernel_spmd(nc, in_maps, core_ids=list(range(NCORES)))
    return _gather(res.results, b_out)
